# revision 11
# baseline (speedup 1.0000x reference)
"""GCN layer (degree-normalized SpMM + dense matmul) on 8 Trainium2 cores.

out = D^-1/2 A D^-1/2 feat W + b, A built from 600K (src, dst) edges.

Sharding: destination nodes across 8 cores (12500 each), re-packed into
98 windows of 128 slots. feat rows are pre-scaled by norm[src] on the
host and cast to bf16.

Gather layout (the key trick): each core stores a PRIVATE permutation of
the feat rows it actually reads (~52.8k distinct of 100k), ordered by
FIRST USE in the core's gather stream and round-robined into 2 banks of
<=BS2 rows (int16-addressable). ~70% of gather descriptors (the
first-use reads) then sweep each bank strictly sequentially - HBM reads
at near-streaming efficiency instead of random-256B efficiency; only
repeat-reads (~30%) stay random. Within each (window, bank) bucket the
slots are sorted by row index so addresses ascend monotonically.

Per window the 2 bank buckets are split so each rounds to few
128-chunks: windows w < W6 get caps (384, 384) -> 6 chunks; the rest
(512, 384) (heavy bank rotating) -> 7. The first-use/bank assignment is
chosen per window to respect the caps and balance bank row counts; the
packer cascades to fewer 6-chunk windows if a core can't fit.

Device pipeline per group of G=7 windows:
  - 2 dma_gather (one per bank; SWDGE queue (g%2)*2+b) pull the group's
    ~5.6k source rows in one instruction each.
  - ONE wide DVE tensor_tensor builds the group's onehot chunks:
    oh[e, k, v] = (iota[v] == dstc[e, k]) in bf16 (stride-0 broadcasts).
  - Per window, TensorE accumulates agg^T[din, v] += X_chunk^T @ oh in
    PSUM (6-7 bf16 matmuls), then psB = outer(1/norm_dst, bias) +
    agg^T^T @ W (bias seeded via a K=1 matmul so the norm_dst scale
    applied next cancels on the bias term).
  - The scalar engine applies norm[dst] on the PSUM->SBUF copy into a
    per-group output tile; one batched DMA writes 7*128 output rows.

Host-side work is shard construction only: degree histograms + rsqrt
norms, node re-packing, first-use bank assignment, edge bucketing,
constant tables, and the inverse node permutation at unshard.
"""

import numpy as np

N_NODES = 100000
N_EDGES = 600000
D = 128
NC = 8            # cores
NPC = 12500       # nodes per core
P = 128           # partitions / window size
W = 98            # windows per core
NB = 2            # feat banks (first-use round-robin)
BS2 = 28672       # bank row capacity (int16-addressable, fixed shape)
G = 7             # windows per gather group (must divide W)
XG_BUFS = 6       # non-rep (graded) build; rep bench uses 3 to fit SBUF

# W6 cascade: number of 6-chunk windows (caps (384,384)); heavier
# windows get (512,384) with the heavy bank rotating by w%2.
W6_LEVELS = [72, 60, 46, 24, 0]


def _caps(w, b, profile):
    """idx slot capacity for (window, bank) at cascade level `profile`."""
    w6 = W6_LEVELS[int(profile)]
    if w < w6:
        return 384
    return 512 if b == (w % NB) else 384


def _build_bass(
    rep=None,
    parts="all",
    bufs=None,
    g_win=None,
    single_packet=None,
    profile=0,
    invn_once=True,
    out_bf16=False,
):
    """parts: 'all' | 'gather' | 'compute' | 'onehot' | 'matmul' isolates
    stages for benchmarking. rep: wrap the group loop in a hardware For_i.
    profile: W6 cascade level index."""
    import concourse.bacc as bacc
    import concourse.bass as bass
    import concourse.mybir as mybir
    import concourse.tile as tile

    f32 = mybir.dt.float32
    bf16 = mybir.dt.bfloat16
    i16 = mybir.dt.int16

    do_gather = parts in ("all", "gather")
    do_onehot = parts in ("all", "compute", "onehot")
    do_matmul = parts in ("all", "compute", "matmul")
    do_tail = parts in ("all", "compute")
    # For_i (rep) mode double-allocates pools for cross-iteration overlap;
    # use shallower buffers there so the bench build fits SBUF.
    XB = bufs or (3 if rep else XG_BUFS)
    OSB = 2 if rep else 4
    OHB = 2 if rep else 3
    GW = g_win or G
    SP = False if single_packet is None else single_packet
    NG = W // GW
    assert NG * GW == W
    PF = profile
    cpw = [sum(_caps(w, b, PF) for b in range(NB)) // 128 for w in range(W)]
    dcol = np.concatenate([[0], np.cumsum(cpw)]).astype(int)  # dstc col offs
    DSTC_COLS = int(dcol[-1])
    # idx col offsets, blocks ordered (bank, window)
    coloff = {}
    acc = 0
    for b in range(NB):
        for w in range(W):
            coloff[(b, w)] = acc
            acc += _caps(w, b, PF) // 16
    IDXC = acc

    nc = bacc.Bacc(
        None,
        target_bir_lowering=False,
        dynamic_dma_scratch_size=32768,
        num_swdge_queues=4,
    )
    feat_b = [
        nc.declare_dram_parameter(f"feat{b}", [BS2, D], bf16, isOutput=False)
        for b in range(NB)
    ]
    w_d = nc.declare_dram_parameter("w", [D, D], bf16, isOutput=False)
    biasrow_d = nc.declare_dram_parameter("biasrow", [1, D], bf16, isOutput=False)
    invn_d = nc.declare_dram_parameter("invn", [1, W * P], bf16, isOutput=False)
    iota_d = nc.declare_dram_parameter("iota", [P, P], bf16, isOutput=False)
    idx_d = nc.declare_dram_parameter("idx", [P, IDXC], i16, isOutput=False)
    dstc_d = nc.declare_dram_parameter("dstc", [P, DSTC_COLS], bf16, isOutput=False)
    normd_d = nc.declare_dram_parameter("normd", [P, W], f32, isOutput=False)
    out_dt = bf16 if out_bf16 else f32
    out_d = nc.declare_dram_parameter("out", [W * P, D], out_dt, isOutput=True)

    with tile.TileContext(nc) as tc:
        with (
            tc.tile_pool(name="const", bufs=1) as cp,
            tc.tile_pool(name="xg", bufs=XB) as xp,
            tc.tile_pool(name="oh", bufs=OHB) as ohp,
            tc.tile_pool(name="sb", bufs=8) as sbp,
            tc.tile_pool(name="osb", bufs=OSB) as obp,
            tc.tile_pool(name="iv", bufs=2) as ivp,
            tc.tile_pool(name="ps1", bufs=4, space="PSUM") as pp1,
            tc.tile_pool(name="ps2", bufs=4, space="PSUM") as pp2,
        ):
            idx_sb = cp.tile([P, IDXC], i16)
            nc.sync.dma_start(out=idx_sb[:], in_=idx_d[:])
            dstc_sb = cp.tile([P, DSTC_COLS], bf16)
            nc.sync.dma_start(out=dstc_sb[:], in_=dstc_d[:])
            normd_sb = cp.tile([P, W], f32)
            nc.sync.dma_start(out=normd_sb[:], in_=normd_d[:])
            iota_sb = cp.tile([P, P], bf16)
            nc.sync.dma_start(out=iota_sb[:], in_=iota_d[:])
            w_sb = cp.tile([D, D], bf16)
            nc.sync.dma_start(out=w_sb[:], in_=w_d[:])
            biasrow_sb = cp.tile([1, D], bf16)
            nc.sync.dma_start(out=biasrow_sb[:], in_=biasrow_d[:])
            if invn_once:
                invn_sb = cp.tile([1, W * P], bf16)
                nc.sync.dma_start(out=invn_sb[:], in_=invn_d[:])

            import contextlib

            loop_cm = tc.For_i(0, rep, 1) if rep else contextlib.nullcontext()
            with loop_cm:
                for g in range(NG):
                    ws = list(range(g * GW, (g + 1) * GW))
                    nch_b = [
                        sum(_caps(w, b, PF) for w in ws) // 128 for b in range(NB)
                    ]
                    bkbase = np.concatenate([[0], np.cumsum(nch_b)]).astype(int)
                    totch = int(bkbase[-1])
                    xg = xp.tile([P, totch * D], bf16, tag="xg")
                    if not do_gather and do_matmul:
                        # benchmark mode: xg needs a writer off the
                        # critical DVE/PE path
                        nc.gpsimd.memset(xg[:], 0.0)
                    for b in (range(NB) if do_gather else []):
                        cs = coloff[(b, g * GW)]
                        nidx = nch_b[b] * 128
                        nc.gpsimd.dma_gather(
                            out_ap=xg[
                                :, int(bkbase[b]) * D : int(bkbase[b + 1]) * D
                            ].rearrange("p (c r) -> p c r", r=D),
                            in_ap=feat_b[b][:, :],
                            idxs_ap=idx_sb[:, cs : cs + nidx // 16],
                            num_idxs=nidx,
                            num_idxs_reg=nidx,
                            elem_size=D,
                            single_packet=SP,
                            queue_num=(g % 2) * 2 + b,
                        )
                    osb = obp.tile([P, GW * D], f32, tag="osb")
                    if do_tail and not invn_once:
                        invn_g = ivp.tile([1, GW * P], bf16, tag="invn")
                        nc.sync.dma_start(
                            out=invn_g[:],
                            in_=invn_d[0:1, g * GW * P : (g + 1) * GW * P],
                        )
                    gch = int(dcol[(g + 1) * GW] - dcol[g * GW])  # == totch
                    ohg = ohp.tile([P, gch * P], bf16, tag="ohg")
                    if do_matmul and not do_onehot:
                        nc.gpsimd.memset(ohg[:], 0.0)
                    if do_onehot:
                        # all the group's onehot chunks in one wide DVE op:
                        # oh[e, k, v] = (iota[v] == dstc[e, k])
                        nc.vector.tensor_tensor(
                            out=ohg[:].rearrange("p (c v) -> p c v", v=P),
                            in0=iota_sb[:].unsqueeze(1).broadcast_to([P, gch, P]),
                            in1=dstc_sb[
                                :, int(dcol[g * GW]) : int(dcol[(g + 1) * GW])
                            ]
                            .unsqueeze(2)
                            .broadcast_to([P, gch, P]),
                            op=mybir.AluOpType.is_equal,
                        )
                    for wl in range(GW):
                        w_i = g * GW + wl
                        psA = pp1.tile([P, P], f32, tag="psA")
                        if do_matmul:
                            wbase = int(dcol[w_i] - dcol[g * GW])
                            cc = 0
                            for b in range(NB):
                                wch = (
                                    sum(_caps(w2, b, PF) for w2 in ws[:wl]) // 128
                                )
                                for j in range(_caps(w_i, b, PF) // 128):
                                    xoff = (int(bkbase[b]) + wch + j) * D
                                    ooff = (wbase + cc) * P
                                    nc.tensor.matmul(
                                        out=psA[:],
                                        lhsT=xg[:, xoff : xoff + D],
                                        rhs=ohg[:, ooff : ooff + P],
                                        start=(cc == 0),
                                        stop=(cc == cpw[w_i] - 1),
                                    )
                                    cc += 1
                        if not do_tail:
                            continue
                        aggT = sbp.tile([P, P], bf16, tag="aggT")
                        nc.scalar.activation(
                            aggT[:], psA[:], mybir.ActivationFunctionType.Copy
                        )
                        psB = pp2.tile([P, D], f32, tag="psB")
                        nc.tensor.matmul(
                            out=psB[:],
                            lhsT=(
                                invn_sb[0:1, w_i * P : (w_i + 1) * P]
                                if invn_once
                                else invn_g[0:1, wl * P : (wl + 1) * P]
                            ),
                            rhs=biasrow_sb[0:1, :],
                            start=True,
                            stop=False,
                        )
                        nc.tensor.matmul(
                            out=psB[:],
                            lhsT=aggT[:],
                            rhs=w_sb[:],
                            start=False,
                            stop=True,
                        )
                        nc.scalar.activation(
                            osb[:, wl * D : (wl + 1) * D],
                            psB[:],
                            mybir.ActivationFunctionType.Copy,
                            scale=normd_sb[:, w_i : w_i + 1],
                        )
                    if do_tail:
                        nc.sync.dma_start(
                            out=out_d[g * GW * P : (g + 1) * GW * P, :].rearrange(
                                "(c p) d -> p c d", p=P
                            ),
                            in_=osb[:].rearrange("p (c d) -> p c d", d=D),
                        )
    nc.compile()
    return nc


def _prep_shards(feat, weight, bias, src, dst, profile=0):
    import ml_dtypes

    bf16 = ml_dtypes.bfloat16
    feat = np.ascontiguousarray(np.asarray(feat, dtype=np.float32))
    weight = np.asarray(weight, dtype=np.float32)
    bias = np.asarray(bias, dtype=np.float32)
    src = np.asarray(src, dtype=np.int64)
    dst = np.asarray(dst, dtype=np.int64)

    deg = np.bincount(dst, minlength=N_NODES)
    norm = (1.0 / np.sqrt(np.maximum(deg, 1.0))).astype(np.float32)
    xs = (feat * norm[:, None]).astype(bf16)  # pre-scaled by norm[src]

    # greedy re-pack of each core's nodes into W windows of <=128 nodes,
    # balancing total edge load against per-window slot capacity targets
    targets = np.array(
        [sum(_caps(w, b, profile) for b in range(NB)) for w in range(W)],
        np.float64,
    )
    dv_all = deg  # in-degree of each node = its slot demand
    slot_of = np.full(N_NODES, -1, np.int32)   # node -> slot (0..127)
    win_of = np.full(N_NODES, -1, np.int32)    # node -> window (0..97)
    perm = np.full((NC, W * P), -1, np.int64)  # (core, w*128+p) -> node
    for m in range(NC):
        nodes = np.arange(m * NPC, (m + 1) * NPC)
        dv = dv_all[nodes].astype(np.float64)
        order = np.argsort(-dv, kind="stable")
        loads = np.zeros(W, np.float64)
        counts = np.zeros(W, np.int32)
        for i in order:
            cand = (loads + dv[i]) / targets
            cand[counts >= P] = np.inf
            w = int(np.argmin(cand))
            n = nodes[i]
            win_of[n] = w
            slot_of[n] = counts[w]
            perm[m, w * P + counts[w]] = n
            loads[w] += dv[i]
            counts[w] += 1

    # --- first-use bank assignment + per-core gather tables ------------
    core_e = dst // NPC
    w_e = win_of[dst]
    cpw = np.array(
        [sum(_caps(w, b, profile) for b in range(NB)) // 128 for w in range(W)]
    )
    dcol = np.concatenate([[0], np.cumsum(cpw)]).astype(np.int64)
    DSTC_COLS = int(dcol[-1])
    coloff = {}
    acc = 0
    for b in range(NB):
        for w in range(W):
            coloff[(b, w)] = acc
            acc += _caps(w, b, profile) // 16
    IDXC = acc

    in_maps = []
    for m in range(NC):
        sel = core_e == m
        es, ed, ew = src[sel], dst[sel], w_e[sel]
        o = np.lexsort((es, ew))  # stream order: window asc, src asc
        es, ed, ew = es[o], ed[o], ew[o]
        fw = np.full(N_NODES, W, np.int32)  # first window using src n
        np.minimum.at(fw, es, ew)
        bank_of = np.full(N_NODES, -1, np.int8)
        row_of = np.full(N_NODES, -1, np.int32)
        rows_b = [[], []]
        ebank = np.empty(len(es), np.int8)
        ok = True
        wstart = np.searchsorted(ew, np.arange(W + 1))
        for w in range(W):
            lo, hi = wstart[w], wstart[w + 1]
            s_w = es[lo:hi]
            cnt = hi - lo
            if cnt == 0:
                continue
            new_mask = fw[s_w] == w
            new_src = np.unique(s_w[new_mask])
            old_bank = bank_of[s_w]
            r0 = int((old_bank == 0).sum())
            r1 = int((old_bank == 1).sum())
            f_edges = cnt - r0 - r1  # edge slots on fresh rows
            cap0, cap1 = _caps(w, 0, profile), _caps(w, 1, profile)
            # choose k = number of fresh rows (ascending src order) sent to
            # bank 0; the rest go to bank 1. Minimize chunk count, then
            # bank row-count imbalance. Vectorized over all k.
            cnt_new = np.bincount(
                np.searchsorted(new_src, s_w[new_mask]), minlength=len(new_src)
            )
            csum = np.concatenate([[0], np.cumsum(cnt_new)])
            a0s = r0 + csum  # bank-0 slots for each k
            a1s = cnt - a0s
            chunks = -(-a0s // 128) + -(-a1s // 128)
            nrow0, nrow1 = len(rows_b[0]), len(rows_b[1])
            ks = np.arange(len(csum))
            bal = np.abs(
                (nrow0 + ks).astype(np.int64)
                - (nrow1 + len(new_src) - ks)
            )
            invalid = (a0s > cap0) | (a1s > cap1)
            score = chunks.astype(np.int64) * (1 << 32) + bal
            score[invalid] = np.iinfo(np.int64).max
            if invalid.all():
                ok = False
                break
            k = int(np.argmin(score))
            g0, g1 = new_src[:k], new_src[k:]
            bank_of[g0] = 0
            bank_of[g1] = 1
            row_of[g0] = len(rows_b[0]) + np.arange(len(g0))
            row_of[g1] = len(rows_b[1]) + np.arange(len(g1))
            rows_b[0].extend(g0.tolist())
            rows_b[1].extend(g1.tolist())
            ebank[lo:hi] = bank_of[s_w]
        if (not ok) or len(rows_b[0]) > BS2 or len(rows_b[1]) > BS2:
            assert profile + 1 < len(W6_LEVELS), "bank overflow at last level"
            return _prep_shards(
                feat, weight, bias, src, dst, profile=profile + 1
            )

        # slot-dense tables [W, NB, cap]; pad idx repeats last real row
        idx_full = np.zeros((IDXC * 16,), np.int16)
        dstc_full = np.full((DSTC_COLS * P,), 255.0, np.float32)
        for w in range(W):
            lo, hi = wstart[w], wstart[w + 1]
            if hi == lo:
                continue
            sw, dw, bw = es[lo:hi], ed[lo:hi], ebank[lo:hi]
            for b in range(NB):
                mb = bw == b
                cap = _caps(w, b, profile)
                rows = row_of[sw[mb]]
                so = np.argsort(rows, kind="stable")
                rows = rows[so]
                slots = slot_of[dw[mb]][so]
                n = len(rows)
                assert n <= cap
                ibase = coloff[(b, w)] * 16
                blk = np.zeros(cap, np.int16)
                blk[:n] = rows.astype(np.int16)
                if n:
                    blk[n:] = rows[-1]
                idx_full[ibase : ibase + cap] = blk
                # dstc columns for this (w, b): chunk-major within window
                chbase = dcol[w] + (
                    _caps(w, 0, profile) // 128 if b else 0
                )
                dblk = np.full(cap, 255.0, np.float32)
                dblk[:n] = slots
                dstc_full[chbase * P : chbase * P + cap] = dblk

        # gather idx layout: per (w,b) block of cap/16 cols, value i at
        # [i%16, i//16]; blocks ordered (bank, window); tiled to 128 parts
        idx16 = np.zeros((16, IDXC), np.int16)
        pos = 0
        for b in range(NB):
            for w in range(W):
                cap = _caps(w, b, profile)
                blk = idx_full[pos * 16 : pos * 16 + cap].reshape(cap // 16, 16)
                idx16[:, pos : pos + cap // 16] = blk.T
                pos += cap // 16
        idx_dev = np.ascontiguousarray(np.tile(idx16, (8, 1)))

        # dstc [128, DSTC_COLS]: column ch holds chunk ch's 128 slot ids
        dstc_dev = np.ascontiguousarray(
            dstc_full.reshape(DSTC_COLS, P).T.astype(bf16)
        )

        bank_arrs = []
        for b in range(NB):
            rb = np.zeros((BS2, D), bf16)
            if rows_b[b]:
                rb[: len(rows_b[b])] = xs[np.asarray(rows_b[b])]
            bank_arrs.append(rb)

        norm_perm = np.where(
            perm[m] >= 0, norm[np.maximum(perm[m], 0)], 0.0
        ).astype(np.float32)
        normd = np.ascontiguousarray(
            norm_perm.reshape(W, P).T.astype(np.float32)
        )  # [128, W]
        invn = np.ascontiguousarray(
            np.where(norm_perm > 0, 1.0 / np.maximum(norm_perm, 1e-30), 0.0)
            .astype(bf16)
            .reshape(1, W * P)
        )

        iota = np.ascontiguousarray(
            np.broadcast_to(np.arange(P, dtype=np.float32), (P, P)).astype(bf16)
        )
        im = {f"feat{b}": bank_arrs[b] for b in range(NB)}
        im.update(
            w=np.ascontiguousarray(weight.astype(bf16)),
            biasrow=np.ascontiguousarray(bias.reshape(1, D).astype(bf16)),
            invn=invn,
            iota=iota,
            idx=idx_dev,
            dstc=dstc_dev,
            normd=normd,
        )
        in_maps.append(im)
    return in_maps, perm, profile


def kernel(feat, weight, bias, src, dst):
    from concourse.bass_utils import run_bass_kernel_spmd

    in_maps, perm, prof = _prep_shards(feat, weight, bias, src, dst)
    nc = _build_bass(profile=prof)
    res = run_bass_kernel_spmd(nc, in_maps, list(range(NC)))
    out = np.empty((N_NODES, D), np.float32)
    for m in range(NC):
        o = np.asarray(res.results[m]["out"], dtype=np.float32)
        mask = perm[m] >= 0
        out[perm[m][mask]] = o[mask]
    return out


# revision 24
# speedup vs baseline: 1.5391x; 1.5391x over previous
"""GCN layer (degree-normalized SpMM + dense matmul) on 8 Trainium2 cores.

out = D^-1/2 A D^-1/2 feat W + b, A built from 600K (src, dst) edges.

Sharding: destination nodes across 8 cores (12500 each), re-packed into
98 windows of 128 slots. feat rows are pre-scaled by norm[src] on the
host and cast to bf16.

Layout (the key trick): gathering one 256B row per edge via SWDGE is
descriptor-throughput-bound (~2.2ns/desc), so descriptors are eliminated
for first-use reads. Each core stores a PRIVATE copy of the rows it
reads, laid out in FIRST-USE ORDER: bank b holds, for every window w, a
fixed-size region of FCAP[b] rows containing the rows whose first read
happens in (w, b) (dead padding rows after them). The ~70% first-use
reads then arrive via ONE contiguous streaming HWDGE DMA per (group,
bank) (big descriptors, line rate, on the Activation ring). Only repeat
reads (~30%, <=128 per (window, bank)) use dma_gather, indexing the
same banks by stored position (int16; late groups use a per-group
in_ap row offset, and repeats that would fall below the offset or
overflow the 128-slot repeat chunk are demoted to duplicated fresh
rows).

Per window: bank0 = 3 fresh chunks + 1 repeat chunk, bank1 = 2 + 1
(7 chunks of 128 slots). Group xg layout: [b0 fresh 7wx3 | b0 rep 7w |
b1 fresh 7wx2 | b1 rep 7w]; dstc columns follow this chunk order, pad
slots carry 255 so the onehot masks them.

Device pipeline per group of G=7 windows:
  - 2 streaming dma_start (ACT ring) pull the fresh regions; 2
    dma_gather (SWDGE queues rotating by group parity) pull repeats.
  - ONE wide DVE tensor_tensor builds the group's onehot chunks:
    oh[e, k, v] = (iota[v] == dstc[e, k]) in bf16 (stride-0 broadcasts).
  - Per window, TensorE accumulates agg^T[din, v] += X_chunk^T @ oh in
    PSUM (7 bf16 matmuls), then psB = outer(1/norm_dst, bias) +
    agg^T^T @ W.
  - The scalar engine applies norm[dst] on the PSUM->SBUF copy; one
    batched DMA (SP ring) writes 7*128 output rows.

Host-side work is shard construction only: degree histograms + rsqrt
norms, node re-packing, first-use layout, edge bucketing, constant
tables, and the inverse node permutation at unshard.
"""

import numpy as np

N_NODES = 100000
N_EDGES = 600000
D = 128
NC = 8            # cores
NPC = 12500       # nodes per core
P = 128           # partitions / window size
W = 98            # windows per core
NB = 2            # banks
G = 7             # windows per gather group (must divide W)
XG_BUFS = 6       # non-rep (graded) build; rep bench uses 3 to fit SBUF

WE = 28                   # early windows: all-fresh (repeats demoted)
FCAP_E = (512, 384)       # fresh rows per (window, bank), w < WE
FCAP_L = (384, 256)       # fresh rows per (window, bank), w >= WE
RCAP = 128                # repeat slots per (window, bank), w >= WE
I16MAX = 32768            # gather idx addressing range per instruction
NGE = WE // G             # early groups (no repeat gathers)


def _fcap(w, b):
    return (FCAP_E if w < WE else FCAP_L)[b]


def _roff(w, b):
    """Row offset of window w's fresh region in bank b."""
    if w <= WE:
        return w * FCAP_E[b]
    return WE * FCAP_E[b] + (w - WE) * FCAP_L[b]


BROWS = (_roff(W, 0), _roff(W, 1))   # bank row counts (41216, 28672)


def _goff(g, b):
    """Gather in_ap row offset for (group, bank): rows referenced are
    < _roff((g+1)*G, b); offset so the int16 idx covers the tail."""
    return max(0, _roff((g + 1) * G, b) - I16MAX)


def _build_bass(
    rep=None,
    parts="all",
    bufs=None,
    ohb=None,
    osbb=None,
    single_packet=False,
    out_bf16=False,
    ring_split=True,  # streams on SP ring (run-ahead); outputs on ACT ring
):
    """parts: 'all' | 'gather' | 'stream' | 'compute' | 'onehot' |
    'matmul' isolates stages for benchmarking. rep: wrap the group loop
    in a hardware For_i."""
    import concourse.bacc as bacc
    import concourse.bass as bass
    import concourse.mybir as mybir
    import concourse.tile as tile

    f32 = mybir.dt.float32
    bf16 = mybir.dt.bfloat16
    i16 = mybir.dt.int16

    do_gather = parts in ("all", "gather")
    do_stream = parts in ("all", "stream", "gather")
    do_onehot = parts in ("all", "compute", "onehot")
    do_matmul = parts in ("all", "compute", "matmul")
    do_tail = parts in ("all", "compute")
    XB = bufs or (3 if rep else XG_BUFS)
    OSB = osbb or (2 if rep else 4)
    OHB = ohb or (2 if rep else 3)
    NG = W // G
    CPW = 7                       # chunks per window
    CPG = G * CPW                 # chunks per group (49)
    # chunk offsets inside a group's xg/dstc space, per group class:
    # early (no repeat chunks): [b0 fresh 7x4 | b1 fresh 7x3]
    # late: [b0 fresh 7x3 | b0 rep 7 | b1 fresh 7x2 | b1 rep 7]
    FCH_E = (FCAP_E[0] // 128, FCAP_E[1] // 128)   # (4, 3)
    FCH_L = (FCAP_L[0] // 128, FCAP_L[1] // 128)   # (3, 2)
    # idx cols per (late group, bank) = G*RCAP/16
    IGB = G * RCAP // 16
    IDXC = (NG - NGE) * NB * IGB

    nc = bacc.Bacc(
        None,
        target_bir_lowering=False,
        dynamic_dma_scratch_size=32768,
        num_swdge_queues=4,
    )
    feat_b = [
        nc.declare_dram_parameter(f"feat{b}", [BROWS[b], D], bf16, isOutput=False)
        for b in range(NB)
    ]
    w_d = nc.declare_dram_parameter("w", [D, D], bf16, isOutput=False)
    biasrow_d = nc.declare_dram_parameter("biasrow", [1, D], bf16, isOutput=False)
    invn_d = nc.declare_dram_parameter("invn", [1, W * P], bf16, isOutput=False)
    iota_d = nc.declare_dram_parameter("iota", [P, P], bf16, isOutput=False)
    idx_d = nc.declare_dram_parameter("idx", [P, IDXC], i16, isOutput=False)
    dstc_d = nc.declare_dram_parameter(
        "dstc", [P, NG * CPG], bf16, isOutput=False
    )
    normd_d = nc.declare_dram_parameter("normd", [P, W], f32, isOutput=False)
    out_dt = bf16 if out_bf16 else f32
    out_d = nc.declare_dram_parameter("out", [W * P, D], out_dt, isOutput=True)

    with tile.TileContext(nc) as tc:
        with (
            tc.tile_pool(name="const", bufs=1) as cp,
            tc.tile_pool(name="xg", bufs=XB) as xp,
            tc.tile_pool(name="oh", bufs=OHB) as ohp,
            tc.tile_pool(name="sb", bufs=8) as sbp,
            tc.tile_pool(name="osb", bufs=OSB) as obp,
            tc.tile_pool(name="ps1", bufs=4, space="PSUM") as pp1,
            tc.tile_pool(name="ps2", bufs=4, space="PSUM") as pp2,
        ):
            idx_sb = cp.tile([P, IDXC], i16)
            nc.sync.dma_start(out=idx_sb[:], in_=idx_d[:])
            dstc_sb = cp.tile([P, NG * CPG], bf16)
            nc.sync.dma_start(out=dstc_sb[:], in_=dstc_d[:])
            normd_sb = cp.tile([P, W], f32)
            nc.sync.dma_start(out=normd_sb[:], in_=normd_d[:])
            iota_sb = cp.tile([P, P], bf16)
            nc.sync.dma_start(out=iota_sb[:], in_=iota_d[:])
            w_sb = cp.tile([D, D], bf16)
            nc.sync.dma_start(out=w_sb[:], in_=w_d[:])
            biasrow_sb = cp.tile([1, D], bf16)
            nc.sync.dma_start(out=biasrow_sb[:], in_=biasrow_d[:])
            invn_sb = cp.tile([1, W * P], bf16)
            nc.sync.dma_start(out=invn_sb[:], in_=invn_d[:])

            import contextlib

            loop_cm = tc.For_i(0, rep, 1) if rep else contextlib.nullcontext()
            with loop_cm:
                for g in range(NG):
                    early = g < NGE
                    FCH = FCH_E if early else FCH_L
                    b0f = 0
                    if early:
                        b1f = G * FCH[0]
                        b0r = b1r = None
                    else:
                        b0r = G * FCH[0]
                        b1f = b0r + G
                        b1r = b1f + G * FCH[1]
                    xg = xp.tile([P, CPG * D], bf16, tag="xg")
                    if not (do_gather or do_stream) and do_matmul:
                        nc.gpsimd.memset(xg[:], 0.0)
                    for b in (range(NB) if do_stream else []):
                        # fresh rows: contiguous streaming load, ACT ring
                        ch0 = b0f if b == 0 else b1f
                        nch = G * FCH[b]
                        eng = nc.sync if ring_split else nc.scalar
                        eng.dma_start(
                            out=xg[:, ch0 * D : (ch0 + nch) * D].rearrange(
                                "p (c d) -> p c d", d=D
                            ),
                            in_=feat_b[b][
                                _roff(g * G, b) : _roff((g + 1) * G, b), :
                            ].rearrange("(c p) d -> p c d", p=P),
                        )
                    for b in (range(NB) if do_gather and not early else []):
                        ch0 = b0r if b == 0 else b1r
                        cs = ((g - NGE) * NB + b) * IGB
                        nidx = G * RCAP
                        nc.gpsimd.dma_gather(
                            out_ap=xg[
                                :, ch0 * D : (ch0 + G) * D
                            ].rearrange("p (c r) -> p c r", r=D),
                            in_ap=feat_b[b][_goff(g, b) :, :],
                            idxs_ap=idx_sb[:, cs : cs + nidx // 16],
                            num_idxs=nidx,
                            num_idxs_reg=nidx,
                            elem_size=D,
                            single_packet=single_packet,
                            queue_num=(g % 2) * 2 + b,
                        )
                    osb = obp.tile([P, G * D], f32, tag="osb")
                    ohg = ohp.tile([P, CPG * P], bf16, tag="ohg")
                    if do_matmul and not do_onehot:
                        nc.gpsimd.memset(ohg[:], 0.0)
                    if do_onehot:
                        # all the group's onehot chunks in one wide DVE op:
                        # oh[e, k, v] = (iota[v] == dstc[e, k])
                        nc.vector.tensor_tensor(
                            out=ohg[:].rearrange("p (c v) -> p c v", v=P),
                            in0=iota_sb[:].unsqueeze(1).broadcast_to([P, CPG, P]),
                            in1=dstc_sb[:, g * CPG : (g + 1) * CPG]
                            .unsqueeze(2)
                            .broadcast_to([P, CPG, P]),
                            op=mybir.AluOpType.is_equal,
                        )
                    for wl in range(G):
                        w_i = g * G + wl
                        # chunk ids of window wl inside the group
                        chunks = [b0f + wl * FCH[0] + j for j in range(FCH[0])]
                        if not early:
                            chunks.append(b0r + wl)
                        chunks += [
                            b1f + wl * FCH[1] + j for j in range(FCH[1])
                        ]
                        if not early:
                            chunks.append(b1r + wl)
                        psA = pp1.tile([P, P], f32, tag="psA")
                        if do_matmul:
                            for cc, ch in enumerate(chunks):
                                nc.tensor.matmul(
                                    out=psA[:],
                                    lhsT=xg[:, ch * D : (ch + 1) * D],
                                    rhs=ohg[:, ch * P : (ch + 1) * P],
                                    start=(cc == 0),
                                    stop=(cc == CPW - 1),
                                )
                        if not do_tail:
                            continue
                        aggT = sbp.tile([P, P], bf16, tag="aggT")
                        nc.scalar.activation(
                            aggT[:], psA[:], mybir.ActivationFunctionType.Copy
                        )
                        psB = pp2.tile([P, D], f32, tag="psB")
                        nc.tensor.matmul(
                            out=psB[:],
                            lhsT=invn_sb[0:1, w_i * P : (w_i + 1) * P],
                            rhs=biasrow_sb[0:1, :],
                            start=True,
                            stop=False,
                        )
                        nc.tensor.matmul(
                            out=psB[:],
                            lhsT=aggT[:],
                            rhs=w_sb[:],
                            start=False,
                            stop=True,
                        )
                        nc.scalar.activation(
                            osb[:, wl * D : (wl + 1) * D],
                            psB[:],
                            mybir.ActivationFunctionType.Copy,
                            scale=normd_sb[:, w_i : w_i + 1],
                        )
                    if do_tail:
                        (nc.scalar if ring_split else nc.sync).dma_start(
                            out=out_d[g * G * P : (g + 1) * G * P, :].rearrange(
                                "(c p) d -> p c d", p=P
                            ),
                            in_=osb[:].rearrange("p (c d) -> p c d", d=D),
                        )
    nc.compile()
    return nc


def _prep_shards(feat, weight, bias, src, dst):
    import ml_dtypes

    bf16 = ml_dtypes.bfloat16
    feat = np.ascontiguousarray(np.asarray(feat, dtype=np.float32))
    weight = np.asarray(weight, dtype=np.float32)
    bias = np.asarray(bias, dtype=np.float32)
    src = np.asarray(src, dtype=np.int64)
    dst = np.asarray(dst, dtype=np.int64)

    deg = np.bincount(dst, minlength=N_NODES)
    norm = (1.0 / np.sqrt(np.maximum(deg, 1.0))).astype(np.float32)
    xs = (feat * norm[:, None]).astype(bf16)  # pre-scaled by norm[src]

    # use-count per node (as gather source) per core, for bank balancing
    # of expected repeat pressure
    core_e = dst // NPC

    # greedy re-pack of each core's nodes into W windows of <=128 nodes,
    # balancing total edge load
    slot_of = np.full(N_NODES, -1, np.int32)
    win_of = np.full(N_NODES, -1, np.int32)
    perm = np.full((NC, W * P), -1, np.int64)
    for m in range(NC):
        nodes = np.arange(m * NPC, (m + 1) * NPC)
        dv = deg[nodes].astype(np.float64)
        order = np.argsort(-dv, kind="stable")
        loads = np.zeros(W, np.float64)
        counts = np.zeros(W, np.int32)
        for i in order:
            cand = loads + dv[i]
            cand[counts >= P] = np.inf
            w = int(np.argmin(cand))
            n = nodes[i]
            win_of[n] = w
            slot_of[n] = counts[w]
            perm[m, w * P + counts[w]] = n
            loads[w] += dv[i]
            counts[w] += 1

    w_e = win_of[dst]
    NGR = W // G
    CPG = G * 7
    FCH_E = (FCAP_E[0] // 128, FCAP_E[1] // 128)
    FCH_L = (FCAP_L[0] // 128, FCAP_L[1] // 128)
    IGB = G * RCAP // 16
    IDXC = (NGR - NGE) * NB * IGB

    def chunk0(w, b, kind):
        """Global dstc/xg chunk id of (window, bank) fresh/rep region."""
        g, wl = w // G, w % G
        early = g < NGE
        FCH = FCH_E if early else FCH_L
        base = g * CPG
        if early:
            assert kind == "f"
            return base + (wl * FCH[0] if b == 0 else G * FCH[0] + wl * FCH[1])
        b0r = G * FCH[0]
        b1f = b0r + G
        b1r = b1f + G * FCH[1]
        if kind == "f":
            return base + (wl * FCH[0] if b == 0 else b1f + wl * FCH[1])
        return base + (b0r + wl if b == 0 else b1r + wl)

    in_maps = []
    for m in range(NC):
        sel = core_e == m
        es, ed, ew = src[sel], dst[sel], w_e[sel]
        o = np.lexsort((es, ew))  # stream order: window asc, src asc
        es, ed, ew = es[o], ed[o], ew[o]
        usecnt = np.bincount(es, minlength=N_NODES)
        fw = np.full(N_NODES, W, np.int32)
        np.minimum.at(fw, es, ew)
        bank_of = np.full(N_NODES, -1, np.int8)
        row_of = np.full(N_NODES, -1, np.int32)  # stored row within bank
        wstart = np.searchsorted(ew, np.arange(W + 1))
        rep_entries = [[[] for b in range(NB)] for w in range(W)]
        fresh_rows = [[[] for b in range(NB)] for w in range(W)]
        fresh_slots = [[[] for b in range(NB)] for w in range(W)]
        repw = np.zeros(NB, np.int64)  # repeat-weight per bank (balance)
        for w in range(W):
            lo, hi = wstart[w], wstart[w + 1]
            if hi == lo:
                continue
            s_w = es[lo:hi]
            d_w = ed[lo:hi]
            uniq, inv = np.unique(s_w, return_inverse=True)
            order2 = np.argsort(inv, kind="stable")
            bnds = np.searchsorted(inv[order2], np.arange(len(uniq) + 1))
            slots_all = slot_of[d_w]
            is_new = fw[uniq] == w
            rcap_w = 0 if w < WE else RCAP
            off_b = [_goff(w // G, b) for b in range(NB)]
            rep_cnt = [0, 0]
            nf = [0, 0]

            def add_fresh(n, sl):
                b = 0 if repw[0] <= repw[1] else 1
                if nf[b] >= _fcap(w, b):
                    b = 1 - b
                assert nf[b] < _fcap(w, b), f"fresh overflow w={w}"
                pos = nf[b]
                nf[b] += 1
                fresh_rows[w][b].append(n)
                fresh_slots[w][b].append(int(sl))
                repw[b] += usecnt[n] - 1
                return b, _roff(w, b) + pos

            # repeats to already-stored rows (or demote: re-store fresh)
            for i in np.nonzero(~is_new)[0]:
                n = uniq[i]
                b = int(bank_of[n])
                r = int(row_of[n])
                ee = order2[bnds[i] : bnds[i + 1]]
                k = len(ee)
                if r >= off_b[b] and rep_cnt[b] + k <= rcap_w:
                    rep_cnt[b] += k
                    for sl in slots_all[ee]:
                        rep_entries[w][b].append((r, int(sl)))
                else:
                    nb_, nr = add_fresh(n, slots_all[ee][0])
                    bank_of[n], row_of[n] = nb_, nr
                    for sl in slots_all[ee][1:]:
                        if rep_cnt[nb_] < rcap_w and nr >= off_b[nb_]:
                            rep_entries[w][nb_].append((nr, int(sl)))
                            rep_cnt[nb_] += 1
                        else:
                            add_fresh(n, sl)  # duplicate row
            # fresh rows
            for i in np.nonzero(is_new)[0]:
                n = uniq[i]
                ee = order2[bnds[i] : bnds[i + 1]]
                b, r = add_fresh(n, slots_all[ee][0])
                bank_of[n], row_of[n] = b, r
                for sl in slots_all[ee][1:]:
                    if rep_cnt[b] < rcap_w and r >= off_b[b]:
                        rep_entries[w][b].append((r, int(sl)))
                        rep_cnt[b] += 1
                    else:
                        add_fresh(n, sl)  # duplicate row

        # --- build device tables ---
        banks = [np.zeros((BROWS[b], D), bf16) for b in range(NB)]
        dstc = np.full((NGR * CPG, P), 255.0, np.float32)
        idx16 = np.zeros((16, IDXC), np.int16)
        for w in range(W):
            gblk = w // G
            wl = w % G
            for b in range(NB):
                fr = fresh_rows[w][b]
                if fr:
                    banks[b][_roff(w, b) : _roff(w, b) + len(fr)] = xs[
                        np.asarray(fr)
                    ]
                ch0 = chunk0(w, b, "f")
                sl = np.asarray(fresh_slots[w][b], np.float32)
                for j in range((len(sl) + 127) // 128):
                    seg = sl[j * 128 : (j + 1) * 128]
                    dstc[ch0 + j, : len(seg)] = seg
                if gblk < NGE:
                    assert not rep_entries[w][b]
                    continue
                # repeats: sort by row for ascending addresses
                ents = rep_entries[w][b]
                rch = chunk0(w, b, "r")
                off = _goff(gblk, b)
                if ents:
                    ents.sort()
                    rr = np.asarray([e[0] for e in ents], np.int64) - off
                    ss = np.asarray([e[1] for e in ents], np.float32)
                    assert rr.min() >= 0 and rr.max() < I16MAX
                    blk = np.full(RCAP, rr[-1], np.int64)
                    blk[: len(rr)] = rr
                    dstc[rch, : len(ss)] = ss
                else:
                    blk = np.zeros(RCAP, np.int64)
                cbase = ((gblk - NGE) * NB + b) * IGB + wl * (RCAP // 16)
                idx16[:, cbase : cbase + RCAP // 16] = (
                    blk.astype(np.int16).reshape(RCAP // 16, 16).T
                )
        idx_dev = np.ascontiguousarray(np.tile(idx16, (8, 1)))
        dstc_dev = np.ascontiguousarray(dstc.T.astype(bf16))

        norm_perm = np.where(
            perm[m] >= 0, norm[np.maximum(perm[m], 0)], 0.0
        ).astype(np.float32)
        normd = np.ascontiguousarray(
            norm_perm.reshape(W, P).T.astype(np.float32)
        )
        invn = np.ascontiguousarray(
            np.where(norm_perm > 0, 1.0 / np.maximum(norm_perm, 1e-30), 0.0)
            .astype(bf16)
            .reshape(1, W * P)
        )
        iota = np.ascontiguousarray(
            np.broadcast_to(np.arange(P, dtype=np.float32), (P, P)).astype(bf16)
        )
        im = {f"feat{b}": banks[b] for b in range(NB)}
        im.update(
            w=np.ascontiguousarray(weight.astype(bf16)),
            biasrow=np.ascontiguousarray(bias.reshape(1, D).astype(bf16)),
            invn=invn,
            iota=iota,
            idx=idx_dev,
            dstc=dstc_dev,
            normd=normd,
        )
        in_maps.append(im)
    return in_maps, perm, 0


def kernel(feat, weight, bias, src, dst):
    from concourse.bass_utils import run_bass_kernel_spmd

    in_maps, perm, _ = _prep_shards(feat, weight, bias, src, dst)
    nc = _build_bass()
    res = run_bass_kernel_spmd(nc, in_maps, list(range(NC)))
    out = np.empty((N_NODES, D), np.float32)
    for m in range(NC):
        o = np.asarray(res.results[m]["out"], dtype=np.float32)
        mask = perm[m] >= 0
        out[perm[m][mask]] = o[mask]
    return out
